# revision 1
# baseline (speedup 1.0000x reference)
"""MeshFC kernel for 8x TRN2 NeuronCores.

Computes: out = inputs @ w + biases, where
  w[i,o] = ||in_pos[i]-out_pos[o]|| - ||init_in_pos[i]-init_out_pos[o]||

Sharding: tensor-parallel on the output dim (8 x 1024 columns). Each core:
  - generates its weight column block on-chip via the PE using the
    augmented-inner-product identity dist^2 = ||a||^2 - 2 a.b + ||b||^2
    (a single K=7 fp32 matmul per tile), sqrt on ScalarE, subtract on DVE
  - runs the main [4096,2048]x[2048,1024] matmul in float32r (FP22)
Host side: pre-transposes/pre-tiles inputs so every DMA is contiguous,
and concatenates the 8 per-core [4096,1024] outputs.
"""

import os
from contextlib import ExitStack

import numpy as np

NUM_IN, NUM_OUT, SD, BATCH = 2048, 8192, 5, 4096
N_CORES = 8
O_SHARD = NUM_OUT // N_CORES  # 1024
B_TILES = BATCH // 128  # 32
K_TILES = NUM_IN // 128  # 16
O_HALves = O_SHARD // 512  # 2

_CACHE = {}


def _build_bass(variant=""):
    import concourse.bass as bass  # noqa: F401
    import concourse.mybir as mybir
    from concourse import bacc
    from concourse.tile import TileContext

    fp32 = mybir.dt.float32
    fp32r = mybir.dt.float32r
    fp16 = mybir.dt.float16

    # main-matmul dtype: fp16 runs at 1 cycle/row (fp32r: 2, fp32: 4+) with
    # accuracy on par with fp22 (10-bit rounded vs 13-bit truncated mantissa)
    mmdt = fp32r if "fp32r" in variant else fp16

    # Bacc (not plain Bass): its compile() runs generate_event_semaphores +
    # move_matmul_waits_to_ldweights, which split multi-waits that exceed the
    # per-instruction HW sync-wait budget.
    nc = bacc.Bacc("TRN2", name="meshfc")

    xT = nc.dram_tensor("xT", [B_TILES, 128, NUM_IN], mmdt, kind="ExternalInput")
    # packed [aC | aI | bC | bI] along the free axis -> single DMA, single wait
    AB_W = 2 * NUM_IN + 2 * O_SHARD
    ab = nc.dram_tensor("ab", [7, AB_W], fp32, kind="ExternalInput")
    # [bias | ones(128)] packed on one partition
    bias = nc.dram_tensor("bias", [1, O_SHARD + 128], mmdt, kind="ExternalInput")
    out = nc.dram_tensor("out", [BATCH, O_SHARD], fp32, kind="ExternalOutput")

    with ExitStack() as ctx:
        tc = ctx.enter_context(TileContext(nc))
        const = ctx.enter_context(tc.tile_pool(name="const", bufs=1))
        wps = mps = tmp = xpool = opool = None
        if "nowgen" not in variant:
            wps = ctx.enter_context(tc.tile_pool(name="wps", bufs=2, space="PSUM"))
            tmp = ctx.enter_context(tc.tile_pool(name="tmp", bufs=2))
        if "nomm" not in variant:
            mps = ctx.enter_context(tc.tile_pool(name="mps", bufs=2, space="PSUM"))
            xpool = ctx.enter_context(tc.tile_pool(name="xp", bufs=3))
            opool = ctx.enter_context(tc.tile_pool(name="op", bufs=3))

        # --- constants ---
        ab_sb = const.tile([7, AB_W], fp32, name="ab_sb")
        nc.sync.dma_start(out=ab_sb, in_=ab[:, :])
        aC_sb = ab_sb[:, 0:NUM_IN]
        aI_sb = ab_sb[:, NUM_IN : 2 * NUM_IN]
        bC_sb = ab_sb[:, 2 * NUM_IN : 2 * NUM_IN + O_SHARD]
        bI_sb = ab_sb[:, 2 * NUM_IN + O_SHARD : AB_W]

        # bias lives on one partition; it is added into PSUM via a K=1 matmul
        biasones_sb = const.tile([1, O_SHARD + 128], mmdt, name="biasones_sb")
        nc.sync.dma_start(out=biasones_sb, in_=bias[:, :])
        bias_sb = biasones_sb[:, 0:O_SHARD]
        ones_sb = biasones_sb[:, O_SHARD : O_SHARD + 128]

        # resident weight block: [128, K_TILES, O_SHARD] = 8 MB
        # float32r so the DVE write rounds to FP22 for the fp32r matmul
        w_sb = const.tile([128, K_TILES, O_SHARD], mmdt, name="w_sb")

        # optional on-device repetition for slope timing (variant "repN")
        n_rep = 1
        for tok in variant.split(","):
            if tok.startswith("rep"):
                n_rep = int(tok[3:])

        # --- weight generation ---
        for _rep in range(n_rep):
            _build_body(nc, tc, variant, const, wps, mps, tmp, xpool, opool,
                        aC_sb, aI_sb, bC_sb, bI_sb, bias_sb, ones_sb, w_sb,
                        xT, out, fp32, mmdt)

    nc.finalize()
    return nc


def _build_body(nc, tc, variant, const, wps, mps, tmp, xpool, opool,
                aC_sb, aI_sb, bC_sb, bI_sb, bias_sb, ones_sb, w_sb,
                xT, out, fp32, mmdt):
    import concourse.mybir as mybir  # noqa: F401

    if True:
        if "nowgen" not in variant:
            for kt in range(K_TILES):
                for oh in range(O_HALves):
                    osl = slice(oh * 512, (oh + 1) * 512)
                    psC = wps.tile([128, 512], fp32, tag="psC", bufs=2)
                    psI = wps.tile([128, 512], fp32, tag="psI", bufs=2)
                    nc.tensor.matmul(
                        psC,
                        aC_sb[:, kt * 128 : (kt + 1) * 128],
                        bC_sb[:, osl],
                        start=True,
                        stop=True,
                    )
                    nc.tensor.matmul(
                        psI,
                        aI_sb[:, kt * 128 : (kt + 1) * 128],
                        bI_sb[:, osl],
                        start=True,
                        stop=True,
                    )
                    # clamp dist^2 to >=0 on DVE (HW fp32 rounding can push
                    # the closest pair slightly negative -> sqrt NaN), then
                    # sqrt in place in SBUF. In-place PSUM activation crashes
                    # the exec unit, so everything lands in SBUF tmps.
                    sC = tmp.tile([128, 512], fp32, tag="sC", bufs=2)
                    sI = tmp.tile([128, 512], fp32, tag="sI", bufs=2)
                    nc.vector.tensor_scalar_max(sC, psC, 0.0)
                    nc.vector.tensor_scalar_max(sI, psI, 0.0)
                    nc.scalar.sqrt(sC, sC)
                    nc.scalar.sqrt(sI, sI)
                    nc.vector.tensor_sub(w_sb[:, kt, osl], sC, sI)

        # --- main matmul: out[b,o] = sum_k x[b,k] w[k,o] (+bias) ---
        if "nomm" in variant:
            return
        for bt in range(B_TILES):
            xt = xpool.tile([128, NUM_IN], mmdt, name="xt")
            if "nodma" not in variant:
                nc.sync.dma_start(out=xt, in_=xT[bt])
            ot = opool.tile([128, O_SHARD], fp32, name="ot")
            # pre-touch: absorbs the out-DMA slot-release wait on ScalarE so
            # the real drains below stay within the HW sync-wait slot limit
            if "nodrain" not in variant:
                nc.scalar.mul(ot[0:1, 0:1], ot[0:1, 0:1], 0.0)
            for oh in range(O_HALves):
                osl = slice(oh * 512, (oh + 1) * 512)
                ps = mps.tile([128, 512], fp32, tag="ps", bufs=2)
                for kt in range(K_TILES):
                    nc.tensor.matmul(
                        ps,
                        xt[:, kt * 128 : (kt + 1) * 128],
                        w_sb[:, kt, osl],
                        start=(kt == 0),
                        stop=("nobias" in variant and kt == K_TILES - 1),
                    )
                # += bias (broadcast over rows via rank-1 matmul)
                if "nobias" not in variant:
                    nc.tensor.matmul(
                        ps, ones_sb[:, :], bias_sb[:, osl], start=False, stop=True
                    )
                if "nodrain" not in variant:
                    nc.scalar.copy(ot[:, osl], ps)
            if "nodrain" not in variant:
                nc.sync.dma_start(out=out[bt * 128 : (bt + 1) * 128, :], in_=ot)


def _prep_inputs(inputs, init_in_pos, init_out_pos, in_pos, out_pos, biases,
                 mm_np_dt=np.float16):
    x = np.ascontiguousarray(np.asarray(inputs, dtype=np.float32))
    a = np.asarray(in_pos, dtype=np.float32).reshape(NUM_IN, SD)
    a0 = np.asarray(init_in_pos, dtype=np.float32).reshape(NUM_IN, SD)
    b = np.asarray(out_pos, dtype=np.float32).reshape(NUM_OUT, SD)
    b0 = np.asarray(init_out_pos, dtype=np.float32).reshape(NUM_OUT, SD)
    bias = np.asarray(biases, dtype=np.float32).reshape(NUM_OUT)

    # [bt, p, kt*128+b'] = x[bt*128+b', kt*128+p]
    xT = np.ascontiguousarray(
        x.reshape(B_TILES, 128, K_TILES, 128).transpose(0, 3, 2, 1).astype(mm_np_dt)
    ).reshape(B_TILES, 128, NUM_IN)

    def aug_a(p):
        return np.concatenate(
            [p.T, (p * p).sum(1)[None, :], np.ones((1, p.shape[0]), np.float32)], 0
        ).astype(np.float32)

    def aug_b(q):
        return np.concatenate(
            [-2.0 * q.T, np.ones((1, q.shape[0]), np.float32), (q * q).sum(1)[None, :]],
            0,
        ).astype(np.float32)

    aCv, aIv = aug_a(a), aug_a(a0)
    bC_full, bI_full = aug_b(b), aug_b(b0)

    in_maps = []
    for c in range(N_CORES):
        sl = slice(c * O_SHARD, (c + 1) * O_SHARD)
        ab = np.ascontiguousarray(
            np.concatenate([aCv, aIv, bC_full[:, sl], bI_full[:, sl]], axis=1)
        )
        in_maps.append(
            {
                "xT": xT,
                "ab": ab,
                "bias": np.ascontiguousarray(
                    np.concatenate([bias[sl], np.ones(128, np.float32)]).astype(
                        mm_np_dt
                    )
                )[None, :],
            }
        )
    return in_maps


def _run(in_maps, trace=False):
    from concourse.bass_utils import run_bass_kernel_spmd

    if "nc" not in _CACHE:
        _CACHE["nc"] = _build_bass()
    nc = _CACHE["nc"]
    res = run_bass_kernel_spmd(
        nc, in_maps, core_ids=list(range(N_CORES)), trace=trace
    )
    outs = [r["out"] for r in res.results]
    return np.concatenate(outs, axis=1), res


def kernel(**inputs) -> np.ndarray:
    in_maps = _prep_inputs(**inputs)
    out, _ = _run(in_maps, trace=bool(os.environ.get("MESHFC_TRACE")))
    return out



# revision 3
# speedup vs baseline: 1.3010x; 1.3010x over previous
"""MeshFC kernel for 8x TRN2 NeuronCores.

Computes: out = inputs @ w + biases, where
  w[i,o] = ||in_pos[i]-out_pos[o]|| - ||init_in_pos[i]-init_out_pos[o]||

Sharding: tensor-parallel on the output dim (8 x 1024 columns). Each core:
  - generates its weight column block on-chip via the PE using the
    augmented-inner-product identity dist^2 = ||a||^2 - 2 a.b + ||b||^2.
    Each fp32 augmented coordinate is split into three bf16 parts
    (8+8+8 mantissa bits >= fp32's 24), and the 6 cross-products that
    matter (hh, hm, mh, hl, lh, mm) become a single K=43 bf16 matmul
    at 1 cycle/row. bf16 products are exact and accumulate in fp32
    PSUM, so dist^2 comes out fp32-accurate (~3e-5 absolute), which
    matters for near-coincident point pairs where sqrt amplifies
    absolute error. (fp32r cannot be used here: the fp32r datapath
    rounds large intermediate products to fp22, giving ~8e-3 error.)
    A small eps coordinate keeps PSUM positive so no DVE clamp is
    needed before sqrt.
  - runs the main [4096,2048]x[2048,1024] matmul in fp16 (1 cycle/row)
Bias is added host-side (it is a free O(N) pass on the gathered output,
vs 64 rank-1 PE matmuls on-device).
Host side: pre-transposes/pre-tiles inputs so every DMA is contiguous,
and concatenates the 8 per-core [4096,1024] outputs.
"""

import os
from contextlib import ExitStack

import numpy as np

NUM_IN, NUM_OUT, SD, BATCH = 2048, 8192, 5, 4096
N_CORES = 8
O_SHARD = NUM_OUT // N_CORES  # 1024
B_TILES = BATCH // 128  # 32
K_TILES = NUM_IN // 128  # 16
O_HALves = O_SHARD // 512  # 2
KAUG = 43  # 7 aug coords x 6 bf16 cross-products + eps coordinate
EPS = 1e-4

_CACHE = {}


def _build_bass(variant=""):
    import concourse.bass as bass  # noqa: F401
    import concourse.mybir as mybir
    from concourse import bacc
    from concourse.tile import TileContext

    fp32 = mybir.dt.float32
    fp32r = mybir.dt.float32r
    bf16 = mybir.dt.bfloat16
    fp16 = mybir.dt.float16

    mmdt = fp32r if "fp32r" in variant else fp16

    nc = bacc.Bacc("TRN2", name="meshfc")

    xT = nc.dram_tensor("xT", [B_TILES, 128, NUM_IN], mmdt, kind="ExternalInput")
    # packed [UC | UI | VC | VI] along the free axis -> single DMA, single wait
    AB_W = 2 * NUM_IN + 2 * O_SHARD
    ab = nc.dram_tensor("ab", [KAUG, AB_W], bf16, kind="ExternalInput")
    out = nc.dram_tensor("out", [BATCH, O_SHARD], fp32, kind="ExternalOutput")

    with ExitStack() as ctx:
        tc = ctx.enter_context(TileContext(nc))
        const = ctx.enter_context(tc.tile_pool(name="const", bufs=1))
        wps = mps = tmp = xpool = opool = None
        if "nowgen" not in variant:
            wps = ctx.enter_context(tc.tile_pool(name="wps", bufs=2, space="PSUM"))
            tmp = ctx.enter_context(tc.tile_pool(name="tmp", bufs=2))
        if "nomm" not in variant:
            mps = ctx.enter_context(tc.tile_pool(name="mps", bufs=2, space="PSUM"))
            xpool = ctx.enter_context(tc.tile_pool(name="xp", bufs=3))
            opool = ctx.enter_context(tc.tile_pool(name="op", bufs=3))

        # --- constants ---
        ab_sb = const.tile([KAUG, AB_W], bf16, name="ab_sb")
        nc.sync.dma_start(out=ab_sb, in_=ab[:, :])
        uC_sb = ab_sb[:, 0:NUM_IN]
        uI_sb = ab_sb[:, NUM_IN : 2 * NUM_IN]
        vC_sb = ab_sb[:, 2 * NUM_IN : 2 * NUM_IN + O_SHARD]
        vI_sb = ab_sb[:, 2 * NUM_IN + O_SHARD : AB_W]

        # resident weight block: [128, K_TILES, O_SHARD] fp16 = 4 MB
        w_sb = const.tile([128, K_TILES, O_SHARD], mmdt, name="w_sb")

        # --- weight generation ---
        if "nowgen" not in variant:
            for kt in range(K_TILES):
                for oh in range(O_HALves):
                    osl = slice(oh * 512, (oh + 1) * 512)
                    ksl = slice(kt * 128, (kt + 1) * 128)
                    psC = wps.tile([128, 512], fp32, tag="psC", bufs=2)
                    psI = wps.tile([128, 512], fp32, tag="psI", bufs=2)
                    nc.tensor.matmul(psC, uC_sb[:, ksl], vC_sb[:, osl],
                                     start=True, stop=True)
                    nc.tensor.matmul(psI, uI_sb[:, ksl], vI_sb[:, osl],
                                     start=True, stop=True)
                    # PSUM values are >= eps^2 - O(1e-6) > 0 by construction,
                    # so sqrt straight out of PSUM; subtract on DVE writes
                    # the fp16 weight tile. (In-place PSUM activation crashes
                    # the exec unit; PSUM->SBUF is fine.)
                    sC = tmp.tile([128, 512], fp32, tag="sC", bufs=2)
                    sI = tmp.tile([128, 512], fp32, tag="sI", bufs=2)
                    nc.scalar.sqrt(sC, psC)
                    nc.scalar.sqrt(sI, psI)
                    nc.vector.tensor_sub(w_sb[:, kt, osl], sC, sI)

        # --- main matmul: out[b,o] = sum_k x[b,k] w[k,o] ---
        if "nomm" not in variant:
            for bt in range(B_TILES):
                xt = xpool.tile([128, NUM_IN], mmdt, name="xt")
                if "nodma" not in variant:
                    nc.sync.dma_start(out=xt, in_=xT[bt])
                ot = opool.tile([128, O_SHARD], fp32, name="ot")
                # pre-touch: absorbs the out-DMA slot-release wait on ScalarE
                # so the real drains below stay within the HW sync-wait slots
                if "nodrain" not in variant:
                    nc.scalar.mul(ot[0:1, 0:1], ot[0:1, 0:1], 0.0)
                for oh in range(O_HALves):
                    osl = slice(oh * 512, (oh + 1) * 512)
                    ps = mps.tile([128, 512], fp32, tag="ps", bufs=2)
                    for kt in range(K_TILES):
                        nc.tensor.matmul(
                            ps,
                            xt[:, kt * 128 : (kt + 1) * 128],
                            w_sb[:, kt, osl],
                            start=(kt == 0),
                            stop=(kt == K_TILES - 1),
                        )
                    if "nodrain" not in variant:
                        nc.scalar.copy(ot[:, osl], ps)
                if "nodrain" not in variant:
                    nc.sync.dma_start(out=out[bt * 128 : (bt + 1) * 128, :], in_=ot)

    nc.finalize()
    return nc


def _split3(a32):
    """Split fp32 -> (hi, mid, lo) bf16 parts with hi+mid+lo == a32 exactly."""
    import ml_dtypes

    bf = ml_dtypes.bfloat16
    h = a32.astype(bf).astype(np.float32)
    m = (a32 - h).astype(bf).astype(np.float32)
    l = (a32 - h - m).astype(bf).astype(np.float32)
    return h, m, l


def _aug_a(p64):  # in-side points [N,5] -> [N,7] fp32 aug
    return np.concatenate(
        [p64, (p64 * p64).sum(1)[:, None], np.ones((len(p64), 1))], 1
    ).astype(np.float32)


def _aug_b(q64):  # out-side points [N,5] -> [N,7] fp32 aug
    return np.concatenate(
        [-2.0 * q64, np.ones((len(q64), 1)), (q64 * q64).sum(1)[:, None]], 1
    ).astype(np.float32)


def _split_u(A):  # [N,7] -> [N,43]: [h,h,m,h,l,m, sqrt(eps)] (pairs w/ _split_v)
    h, m, l = _split3(A)
    e = np.full((len(A), 1), np.sqrt(EPS), np.float32)
    return np.concatenate([h, h, m, h, l, m, e], 1)


def _split_v(B):  # [N,7] -> [N,43]: [h,m,h,l,h,m, sqrt(eps)]
    h, m, l = _split3(B)
    e = np.full((len(B), 1), np.sqrt(EPS), np.float32)
    return np.concatenate([h, m, h, l, h, m, e], 1)


def _prep_inputs(inputs, init_in_pos, init_out_pos, in_pos, out_pos, biases,
                 mm_np_dt=np.float16):
    x = np.ascontiguousarray(np.asarray(inputs, dtype=np.float32))
    a = np.asarray(in_pos, dtype=np.float64).reshape(NUM_IN, SD)
    a0 = np.asarray(init_in_pos, dtype=np.float64).reshape(NUM_IN, SD)
    b = np.asarray(out_pos, dtype=np.float64).reshape(NUM_OUT, SD)
    b0 = np.asarray(init_out_pos, dtype=np.float64).reshape(NUM_OUT, SD)
    bias = np.asarray(biases, dtype=np.float32).reshape(NUM_OUT)

    # [bt, p, kt*128+b'] = x[bt*128+b', kt*128+p]
    xT = np.ascontiguousarray(
        x.reshape(B_TILES, 128, K_TILES, 128).transpose(0, 3, 2, 1).astype(mm_np_dt)
    ).reshape(B_TILES, 128, NUM_IN)

    uC = _split_u(_aug_a(a)).T  # [22, 2048]
    uI = _split_u(_aug_a(a0)).T
    vC_full = _split_v(_aug_b(b)).T  # [22, 8192]
    vI_full = _split_v(_aug_b(b0)).T

    in_maps = []
    for c in range(N_CORES):
        sl = slice(c * O_SHARD, (c + 1) * O_SHARD)
        import ml_dtypes

        ab = np.ascontiguousarray(
            np.concatenate([uC, uI, vC_full[:, sl], vI_full[:, sl]], axis=1)
        ).astype(ml_dtypes.bfloat16)
        in_maps.append({"xT": xT, "ab": ab})
    return in_maps, bias


def _run(in_maps, trace=False):
    from concourse.bass_utils import run_bass_kernel_spmd

    if "nc" not in _CACHE:
        _CACHE["nc"] = _build_bass()
    nc = _CACHE["nc"]
    res = run_bass_kernel_spmd(
        nc, in_maps, core_ids=list(range(N_CORES)), trace=trace
    )
    outs = [r["out"] for r in res.results]
    return np.concatenate(outs, axis=1), res


def kernel(**inputs) -> np.ndarray:
    in_maps, bias = _prep_inputs(**inputs)
    out, _ = _run(in_maps, trace=bool(os.environ.get("MESHFC_TRACE")))
    return out + bias[None, :]


# revision 4
# speedup vs baseline: 1.3722x; 1.0548x over previous
"""MeshFC kernel for 8x TRN2 NeuronCores.

Computes: out = inputs @ w + biases, where
  w[i,o] = ||in_pos[i]-out_pos[o]|| - ||init_in_pos[i]-init_out_pos[o]||

Sharding: tensor-parallel on the output dim (8 x 1024 columns). Each core:
  - generates its weight column block on-chip via the PE using the
    augmented-inner-product identity dist^2 = ||a||^2 - 2 a.b + ||b||^2.
    Each fp32 augmented coordinate is split into two fp16 parts
    (11+11 mantissa bits), and the cross-products (hh, hm, mh, mm)
    become a single K=29 fp16 matmul at 1 cycle/row. fp16 products are
    exact and accumulate in fp32 PSUM, so dist^2 comes out accurate to
    ~2e-5 absolute, which matters for near-coincident point pairs
    where sqrt amplifies absolute error. (fp32r cannot be used here:
    the fp32r datapath rounds large intermediate products to fp22,
    giving ~8e-3 error; bf16 measured 2 cycles/row on HW.)
    A small eps coordinate keeps PSUM positive so no DVE clamp is
    needed before sqrt.
  - runs the main [4096,2048]x[2048,1024] matmul in fp16 (1 cycle/row)
Bias is added host-side (it is a free O(N) pass on the gathered output,
vs 64 rank-1 PE matmuls on-device).
Host side: pre-transposes/pre-tiles inputs so every DMA is contiguous,
and concatenates the 8 per-core [4096,1024] outputs.
"""

import os
from contextlib import ExitStack

import numpy as np

NUM_IN, NUM_OUT, SD, BATCH = 2048, 8192, 5, 4096
N_CORES = 8
O_SHARD = NUM_OUT // N_CORES  # 1024
B_TILES = BATCH // 128  # 32
K_TILES = NUM_IN // 128  # 16
O_HALves = O_SHARD // 512  # 2
KAUG = 29  # 7 aug coords x 4 fp16 cross-products + eps coordinate
EPS = 1e-4

_CACHE = {}


def _build_bass(variant=""):
    import concourse.bass as bass  # noqa: F401
    import concourse.mybir as mybir
    from concourse import bacc
    from concourse.tile import TileContext

    fp32 = mybir.dt.float32
    fp32r = mybir.dt.float32r
    bf16 = mybir.dt.bfloat16
    fp16 = mybir.dt.float16

    mmdt = fp32r if "fp32r" in variant else fp16

    nc = bacc.Bacc("TRN2", name="meshfc")

    xT = nc.dram_tensor("xT", [B_TILES, 128, NUM_IN], mmdt, kind="ExternalInput")
    # packed [UC | UI | VC | VI] along the free axis -> single DMA, single wait
    AB_W = 2 * NUM_IN + 2 * O_SHARD
    ab = nc.dram_tensor("ab", [KAUG, AB_W], fp16, kind="ExternalInput")
    out = nc.dram_tensor("out", [BATCH, O_SHARD], fp32, kind="ExternalOutput")

    with ExitStack() as ctx:
        tc = ctx.enter_context(TileContext(nc))
        const = ctx.enter_context(tc.tile_pool(name="const", bufs=1))
        wps = mps = tmp = xpool = opool = None
        if "nowgen" not in variant:
            wps = ctx.enter_context(tc.tile_pool(name="wps", bufs=2, space="PSUM"))
            tmp = ctx.enter_context(tc.tile_pool(name="tmp", bufs=2))
        if "nomm" not in variant:
            mps = ctx.enter_context(tc.tile_pool(name="mps", bufs=2, space="PSUM"))
            xpool = ctx.enter_context(tc.tile_pool(name="xp", bufs=3))
            opool = ctx.enter_context(tc.tile_pool(name="op", bufs=3))

        # --- constants ---
        ab_sb = const.tile([KAUG, AB_W], fp16, name="ab_sb")
        # chunk by partition ranges: each partition row is one ~12KB DMA
        # descriptor and a single queue moves only ~26 GB/s, so one big
        # dma_start serializes ~20us; 4-partition chunks fan out across
        # queues and land in ~2us.
        for p0 in range(0, KAUG, 4):
            p1 = min(p0 + 4, KAUG)
            nc.sync.dma_start(out=ab_sb[p0:p1, :], in_=ab[p0:p1, :])
        uC_sb = ab_sb[:, 0:NUM_IN]
        uI_sb = ab_sb[:, NUM_IN : 2 * NUM_IN]
        vC_sb = ab_sb[:, 2 * NUM_IN : 2 * NUM_IN + O_SHARD]
        vI_sb = ab_sb[:, 2 * NUM_IN + O_SHARD : AB_W]

        # resident weight block: [128, K_TILES, O_SHARD] fp16 = 4 MB
        w_sb = const.tile([128, K_TILES, O_SHARD], mmdt, name="w_sb")

        # --- weight generation ---
        if "nowgen" not in variant:
            for kt in range(K_TILES):
                for oh in range(O_HALves):
                    osl = slice(oh * 512, (oh + 1) * 512)
                    ksl = slice(kt * 128, (kt + 1) * 128)
                    psC = wps.tile([128, 512], fp32, tag="psC", bufs=2)
                    psI = wps.tile([128, 512], fp32, tag="psI", bufs=2)
                    nc.tensor.matmul(psC, uC_sb[:, ksl], vC_sb[:, osl],
                                     start=True, stop=True)
                    nc.tensor.matmul(psI, uI_sb[:, ksl], vI_sb[:, osl],
                                     start=True, stop=True)
                    # PSUM values are >= eps^2 - O(1e-6) > 0 by construction,
                    # so sqrt straight out of PSUM; subtract on DVE writes
                    # the fp16 weight tile. (In-place PSUM activation crashes
                    # the exec unit; PSUM->SBUF is fine.)
                    sC = tmp.tile([128, 512], fp32, tag="sC", bufs=2)
                    sI = tmp.tile([128, 512], fp32, tag="sI", bufs=2)
                    nc.scalar.sqrt(sC, psC)
                    nc.scalar.sqrt(sI, psI)
                    nc.vector.tensor_sub(w_sb[:, kt, osl], sC, sI)

        # --- main matmul: out[b,o] = sum_k x[b,k] w[k,o] ---
        if "nomm" not in variant:
            for bt in range(B_TILES):
                xt = xpool.tile([128, NUM_IN], mmdt, name="xt")
                if "nodma" not in variant:
                    nc.sync.dma_start(out=xt, in_=xT[bt])
                ot = opool.tile([128, O_SHARD], fp32, name="ot")
                # pre-touch: absorbs the out-DMA slot-release wait on ScalarE
                # so the real drains below stay within the HW sync-wait slots
                if "nodrain" not in variant:
                    nc.scalar.mul(ot[0:1, 0:1], ot[0:1, 0:1], 0.0)
                for oh in range(O_HALves):
                    osl = slice(oh * 512, (oh + 1) * 512)
                    ps = mps.tile([128, 512], fp32, tag="ps", bufs=2)
                    for kt in range(K_TILES):
                        nc.tensor.matmul(
                            ps,
                            xt[:, kt * 128 : (kt + 1) * 128],
                            w_sb[:, kt, osl],
                            start=(kt == 0),
                            stop=(kt == K_TILES - 1),
                        )
                    if "nodrain" not in variant:
                        nc.scalar.copy(ot[:, osl], ps)
                if "nodrain" not in variant:
                    nc.sync.dma_start(out=out[bt * 128 : (bt + 1) * 128, :], in_=ot)

    nc.finalize()
    return nc


def _split2(a32):
    """Split fp32 -> (hi, mid) fp16 parts; hi+mid covers 22 mantissa bits."""
    h = a32.astype(np.float16).astype(np.float32)
    m = (a32 - h).astype(np.float16).astype(np.float32)
    return h, m


def _aug_a(p64):  # in-side points [N,5] -> [N,7] fp32 aug
    return np.concatenate(
        [p64, (p64 * p64).sum(1)[:, None], np.ones((len(p64), 1))], 1
    ).astype(np.float32)


def _aug_b(q64):  # out-side points [N,5] -> [N,7] fp32 aug
    return np.concatenate(
        [-2.0 * q64, np.ones((len(q64), 1)), (q64 * q64).sum(1)[:, None]], 1
    ).astype(np.float32)


def _split_u(A):  # [N,7] -> [N,29]: [h,h,m,m, sqrt(eps)] (pairs w/ _split_v)
    h, m = _split2(A)
    e = np.full((len(A), 1), np.sqrt(EPS), np.float32)
    return np.concatenate([h, h, m, m, e], 1)


def _split_v(B):  # [N,7] -> [N,29]: [h,m,h,m, sqrt(eps)]
    h, m = _split2(B)
    e = np.full((len(B), 1), np.sqrt(EPS), np.float32)
    return np.concatenate([h, m, h, m, e], 1)


def _prep_inputs(inputs, init_in_pos, init_out_pos, in_pos, out_pos, biases,
                 mm_np_dt=np.float16):
    x = np.ascontiguousarray(np.asarray(inputs, dtype=np.float32))
    a = np.asarray(in_pos, dtype=np.float64).reshape(NUM_IN, SD)
    a0 = np.asarray(init_in_pos, dtype=np.float64).reshape(NUM_IN, SD)
    b = np.asarray(out_pos, dtype=np.float64).reshape(NUM_OUT, SD)
    b0 = np.asarray(init_out_pos, dtype=np.float64).reshape(NUM_OUT, SD)
    bias = np.asarray(biases, dtype=np.float32).reshape(NUM_OUT)

    # [bt, p, kt*128+b'] = x[bt*128+b', kt*128+p]
    xT = np.ascontiguousarray(
        x.reshape(B_TILES, 128, K_TILES, 128).transpose(0, 3, 2, 1).astype(mm_np_dt)
    ).reshape(B_TILES, 128, NUM_IN)

    uC = _split_u(_aug_a(a)).T  # [22, 2048]
    uI = _split_u(_aug_a(a0)).T
    vC_full = _split_v(_aug_b(b)).T  # [22, 8192]
    vI_full = _split_v(_aug_b(b0)).T

    in_maps = []
    for c in range(N_CORES):
        sl = slice(c * O_SHARD, (c + 1) * O_SHARD)
        ab = np.ascontiguousarray(
            np.concatenate([uC, uI, vC_full[:, sl], vI_full[:, sl]], axis=1)
        ).astype(np.float16)
        in_maps.append({"xT": xT, "ab": ab})
    return in_maps, bias


def _run(in_maps, trace=False):
    from concourse.bass_utils import run_bass_kernel_spmd

    if "nc" not in _CACHE:
        _CACHE["nc"] = _build_bass()
    nc = _CACHE["nc"]
    res = run_bass_kernel_spmd(
        nc, in_maps, core_ids=list(range(N_CORES)), trace=trace
    )
    outs = [r["out"] for r in res.results]
    return np.concatenate(outs, axis=1), res


def kernel(**inputs) -> np.ndarray:
    in_maps, bias = _prep_inputs(**inputs)
    out, _ = _run(in_maps, trace=bool(os.environ.get("MESHFC_TRACE")))
    return out + bias[None, :]
